# revision 2
# baseline (speedup 1.0000x reference)
"""CenterLoss kernel for Trainium2 (8 NeuronCores, Bass).

Reference computation:
    c    = centers[labels]              # [B, D] gather (B=256, D=512)
    dist = sum((x - c)**2, axis=1)      # [B]
    dist = clip(dist, 1e-12, 1e12)
    out  = mean(dist)                   # scalar f32

Sharding strategy (per the class of "gather the needed B rows" schemes):
  - The gather of the B=256 needed center rows out of the 85742-row table is
    pure data movement; it is done host-side while building each core's input
    shard (equivalent of the "all-gather the needed B rows" plan).
  - Batch is sharded 32 rows/core across 8 cores. Each core's [32, 512] x and
    c shards are laid out as [128, 128] SBUF tiles (each batch row occupies 4
    partitions), so the DVE runs at full 128-lane width.
  - Device computes d = x - c (one DVE op) then d*d with a fused per-partition
    row-sum (one DVE tensor_tensor_reduce op) -> [128, 1] partial sums.
  - Host sums each row's 4 partials, applies the clip, and takes the mean
    (256 scalars; the all-reduce/unshard step).
"""

import numpy as np

import concourse.bass as bass
import concourse.mybir as mybir
from concourse.bass_utils import run_bass_kernel_spmd

B = 256
D = 512
N_CORES = 8
ROWS_PER_CORE = B // N_CORES          # 32
P = 128                               # SBUF partitions
FREE = ROWS_PER_CORE * D // P         # 128 free elements per partition
SPLIT = D // FREE                     # 4 partitions per batch row

_nc_cache = None


def _build_nc() -> bass.Bass:
    nc = bass.Bass()
    f32 = mybir.dt.float32

    xs = nc.dram_tensor("xs", [P, FREE], f32, kind="ExternalInput")
    cs = nc.dram_tensor("cs", [P, FREE], f32, kind="ExternalInput")
    out = nc.dram_tensor("partial", [P, 1], f32, kind="ExternalOutput")

    with (
        nc.sbuf_tensor([P, FREE], f32) as xt,
        nc.sbuf_tensor([P, FREE], f32) as ct,
        nc.sbuf_tensor([P, FREE], f32) as dt,
        nc.sbuf_tensor([P, FREE], f32) as sq,
        nc.sbuf_tensor([P, 1], f32) as rt,
        nc.semaphore("dsem") as dsem,
        nc.semaphore("vsem") as vsem,
        nc.Block() as block,
    ):

        @block.sync
        def _(sync):
            sync.dma_start(out=xt[:], in_=xs[:]).then_inc(dsem, 16)
            sync.dma_start(out=ct[:], in_=cs[:]).then_inc(dsem, 16)
            sync.wait_ge(vsem, 1)
            sync.dma_start(out=out[:], in_=rt[:]).then_inc(dsem, 16)
            sync.wait_ge(dsem, 48)

        @block.vector
        def _(vector):
            vector.wait_ge(dsem, 32)
            vector.tensor_sub(dt[:], xt[:], ct[:])
            vector.scalar_tensor_tensor(
                out=sq[:],
                in0=dt[:],
                scalar=0.0,
                in1=dt[:],
                op0=mybir.AluOpType.bypass,
                op1=mybir.AluOpType.mult,
                accum_out=rt[:],
            ).then_inc(vsem, 1)

    return nc


def kernel(x: np.ndarray, labels: np.ndarray, centers: np.ndarray) -> np.ndarray:
    global _nc_cache
    x = np.ascontiguousarray(np.asarray(x, dtype=np.float32))
    labels = np.asarray(labels)
    centers = np.asarray(centers, dtype=np.float32)

    c = np.ascontiguousarray(centers[labels])          # [B, D] host-side gather

    xr = x.reshape(N_CORES, P, FREE)
    cr = c.reshape(N_CORES, P, FREE)
    in_maps = [{"xs": xr[i], "cs": cr[i]} for i in range(N_CORES)]

    if _nc_cache is None:
        _nc_cache = _build_nc()

    res = run_bass_kernel_spmd(_nc_cache, in_maps, core_ids=list(range(N_CORES)))

    partials = np.stack([res.results[i]["partial"][:, 0] for i in range(N_CORES)])
    dist = partials.reshape(B, SPLIT).astype(np.float64).sum(axis=1)
    dist = np.clip(dist, 1e-12, 1e12)
    return np.asarray(dist.mean(), dtype=np.float32)


# revision 3
# speedup vs baseline: 1.3557x; 1.3557x over previous
"""CenterLoss kernel for Trainium2 (8 NeuronCores, Bass).

Reference computation:
    c    = centers[labels]              # [B, D] gather (B=256, D=512)
    dist = sum((x - c)**2, axis=1)      # [B]
    dist = clip(dist, 1e-12, 1e12)
    out  = mean(dist)                   # scalar f32

Sharding strategy (the "all-gather the needed B rows" plan):
  - The gather of the B=256 needed center rows out of the large table is pure
    data movement; it is done host-side while building each core's input shard.
  - Batch is sharded 32 rows/core across 8 cores.
  - Per-core layout is feature-transposed: partition p holds feature
    (chunk*128 + p) and free position r*4 + chunk holds batch row r, so the
    chunk-reduce over 128 partitions can run on the PE at full width.
    x, c, and a ones-column travel in ONE [128, 257] DMA.
  - Device: d = x - c; sq = d*d (DVE, full 128-lane width); column-sum of sq
    via PE matmul with a ones vector -> PSUM [1, 128]; DVE reduces each row's
    4 chunk-sums -> [1, 32] per-row squared distances; single-packet DMA out.
  - Host applies the clip and the mean (256 scalars; the all-reduce step).

Hard-won correctness rules baked in here:
  - The sync engine MUST wait for the output DMA's completion semaphore before
    falling into the end-of-program barrier, or the runtime reads back the
    output buffer while the write tail is still in flight (the last ~1/8 of
    the buffer arrives stale).
  - Dependent same-engine DVE ops pipeline under relaxed ordering with a fixed
    issue lag; a short consumer overtakes a long producer and reads its tail
    before it is written. Equal-length streaming pairs (sub -> mul) are safe;
    the final reduce is the last DVE op and is only read by the out-DMA
    ~600ns later, which is safe. No short op may follow the reduce.
  - PE matmul completion does not imply PSUM write visibility; drain the PE
    pipe before releasing the DVE read of PSUM.
"""

import numpy as np

import concourse.bass as bass
import concourse.mybir as mybir
from concourse.bass_utils import run_bass_kernel_spmd

B = 256
D = 512
N_CORES = 8
P = 128                               # SBUF partitions
R = B // N_CORES                      # 32 batch rows per core
CH = D // P                           # 4 feature chunks per row
F = CH * R                            # 128 free elements per partition

_nc_cache = None


def _build_nc() -> bass.Bass:
    nc = bass.Bass()
    f32 = mybir.dt.float32

    big = nc.dram_tensor("big", [P, 2 * F + 1], f32, kind="ExternalInput")
    out = nc.dram_tensor("dist", [1, R], f32, kind="ExternalOutput")

    with (
        nc.sbuf_tensor([P, 2 * F + 1], f32) as bs,
        nc.sbuf_tensor([P, F], f32) as dt,
        nc.sbuf_tensor([P, F], f32) as sq,
        nc.sbuf_tensor([1, R], f32) as dist_raw,
        nc.psum_tensor([1, F], f32) as ps,
        nc.semaphore("dsem") as dsem,
        nc.semaphore("vsem") as vsem,
        nc.semaphore("tsem") as tsem,
        nc.semaphore("osem") as osem,
        nc.Block() as block,
    ):
        xt = bs[:, 0:F]
        ct = bs[:, F:2 * F]
        ones = bs[:, 2 * F:2 * F + 1]

        @block.sync
        def _(sync):
            sync.dma_start(out=bs[:], in_=big[:]).then_inc(dsem, 16)
            sync.wait_ge(osem, 1)
            sync.dma_start(out=out[:], in_=dist_raw[:]).then_inc(dsem, 16)
            sync.wait_ge(dsem, 32)

        @block.tensor
        def _(tensor):
            tensor.wait_ge(vsem, 1)
            tensor.matmul(ps[:], ones, sq[:], start=True, stop=True)
            tensor.drain().then_inc(tsem, 1)

        @block.vector
        def _(vector):
            vector.wait_ge(dsem, 16)
            vector.tensor_sub(dt[:], xt, ct)
            vector.tensor_mul(sq[:], dt[:], dt[:]).then_inc(vsem, 1)
            vector.wait_ge(tsem, 1)
            view = ps[:].rearrange("p (r c) -> p r c", c=CH, r=R)
            vector.tensor_reduce(
                dist_raw[:], view, axis=mybir.AxisListType.X,
                op=mybir.AluOpType.add,
            ).then_inc(osem, 1)

    return nc


def _transpose_shard(a: np.ndarray) -> np.ndarray:
    # [R, D] -> [P, F]: partition = feature-within-chunk, free = row*CH + chunk
    return np.ascontiguousarray(
        a.T.reshape(CH, P, R).transpose(1, 2, 0).reshape(P, F)
    )


def kernel(x: np.ndarray, labels: np.ndarray, centers: np.ndarray) -> np.ndarray:
    global _nc_cache
    x = np.asarray(x, dtype=np.float32)
    labels = np.asarray(labels)
    centers = np.asarray(centers, dtype=np.float32)

    c = centers[labels]                                # [B, D] host-side gather

    ones = np.ones((P, 1), np.float32)
    in_maps = []
    for i in range(N_CORES):
        xs = _transpose_shard(x[i * R:(i + 1) * R])
        cs = _transpose_shard(c[i * R:(i + 1) * R])
        in_maps.append(
            {"big": np.ascontiguousarray(np.concatenate([xs, cs, ones], axis=1))}
        )

    if _nc_cache is None:
        _nc_cache = _build_nc()

    res = run_bass_kernel_spmd(_nc_cache, in_maps, core_ids=list(range(N_CORES)))

    dist = np.concatenate(
        [res.results[i]["dist"][0].astype(np.float64) for i in range(N_CORES)]
    )
    dist = np.clip(dist, 1e-12, 1e12)
    return np.asarray(dist.mean(), dtype=np.float32)
